# revision 6
# baseline (speedup 1.0000x reference)
"""DualAttention (cross+self bidirectional attention, 2 streams) on 8 TRN2 cores.

Sharding: data-parallel over batch (4) x tensor-parallel over heads (2 groups
of 8 heads). Core c handles batch c//2, head-group c%2. Each core computes its
head-group's slice of all 6 input projections, RoPE, the 4 attention combos,
and a partial output projection; the host sums the two partial out-projections
per batch and adds the output bias.

Key device-side layout choices:
  - q/k projections computed TRANSPOSED ([e_out, s]) with rot-grouped row
    permutation so RoPE's rotate_half becomes a fixed 128x128 permutation
    matmul + elementwise combines, and attention scores can be computed as
    S^T = K^T(d,sk)^T-contracted-with-Q^T(d,sq) using K=32 row-packed matmuls.
  - keys are pre-packed on the host (masked keys dropped, padded to SKP), so
    softmax masking reduces to a per-partition bias of -30000 on pad rows and
    the whole attention is ~2x smaller.
  - softmax is unnormalized: U = exp(scale*S^T + maskbias); row sums r come
    from ones-lhsT matmuls; normalization multiplies O^T by replicated 1/r.
  - matmuls run in float32r (tf32-class, full PE rate); U/V/att are bf16.
"""

import numpy as np

B, S, E, H = 4, 1024, 1024, 16
D, R = 64, 32
HG, EG = 8, 512  # heads / e-columns per head-group
P = 128
KE = E // P  # contraction chunks of a projection
SCALE = D ** -0.5
NCORES = 8
MASK_BIAS = -30000.0

_PROG_CACHE = {}


def _rot_perm():
    """Row order: [h0..h3 rot | h4..h7 rot | h0..h3 pass | h4..h7 pass]."""
    idx = []
    for blk in range(2):
        for j in range(4):
            h = 4 * blk + j
            idx += [64 * h + d for d in range(32)]
    for blk in range(2):
        for j in range(4):
            h = 4 * blk + j
            idx += [64 * h + d for d in range(32, 64)]
    return np.array(idx, np.int64)


def _build_program(SKP, reps=1):
    key = (SKP, reps)
    if key in _PROG_CACHE:
        return _PROG_CACHE[key]

    import concourse.bass as bass
    import concourse.tile as tile
    from concourse import bacc, mybir
    from contextlib import ExitStack

    f32 = mybir.dt.float32
    f32r = mybir.dt.float32r
    bf16 = mybir.dt.bfloat16
    NKC = SKP // P
    ts = bass.ts

    nc = bacc.Bacc("TRN2", target_bir_lowering=False, debug=False, num_devices=NCORES)

    def din(name, shape, dt):
        return nc.dram_tensor(name, list(shape), dt, kind="ExternalInput").ap()

    a = {}
    a["x1T"] = din("x1T", (E, S), f32r)
    a["x2T"] = din("x2T", (E, S), f32r)
    a["x1p"] = din("x1p", (E, SKP), f32r)
    a["x2p"] = din("x2p", (E, SKP), f32r)
    for n in ("q1", "q2", "k1", "k2", "v1", "v2"):
        a["W" + n] = din("W" + n, (E, EG), f32r)
    a["Wo1"] = din("Wo1", (EG, S), bf16)
    a["Wo2"] = din("Wo2", (EG, S), bf16)
    for n in ("q1", "q2", "k1", "k2"):
        a["b" + n] = din("b" + n, (P, 4), f32)
    a["bv1"] = din("bv1", (P, EG), f32)
    a["bv2"] = din("bv2", (P, EG), f32)
    for n in ("cq1", "sq1", "cq2", "sq2"):
        a[n] = din(n, (P, S), bf16)
    for n in ("ck1", "sk1", "ck2", "sk2"):
        a[n] = din(n, (P, SKP), bf16)
    a["mb1"] = din("mb1", (P, NKC), f32)
    a["mb2"] = din("mb2", (P, NKC), f32)
    a["perm"] = din("perm", (P, P), f32r)
    a["selmat"] = din("selmat", (P, P), f32r)
    a["zeros"] = din("zeros", (P, 512), f32r)
    out1 = nc.dram_tensor("o1", [S, E], f32, kind="ExternalOutput").ap()
    out2 = nc.dram_tensor("o2", [S, E], f32, kind="ExternalOutput").ap()

    Exp = mybir.ActivationFunctionType.Exp

    def emit(tc):
        with ExitStack() as ctx:
            consts = ctx.enter_context(tc.tile_pool(name="consts", bufs=1))
            xch = ctx.enter_context(tc.tile_pool(name="xch", bufs=3))
            xpp = ctx.enter_context(tc.tile_pool(name="xpp", bufs=1))
            wpool = ctx.enter_context(tc.tile_pool(name="wpool", bufs=2))
            qkv = ctx.enter_context(tc.tile_pool(name="qkv", bufs=1))
            attp = ctx.enter_context(tc.tile_pool(name="attp", bufs=1))
            upool = ctx.enter_context(tc.tile_pool(name="upool", bufs=3))
            sm = ctx.enter_context(tc.tile_pool(name="sm", bufs=2))
            outp = ctx.enter_context(tc.tile_pool(name="outp", bufs=2))
            bigps = ctx.enter_context(tc.tile_pool(name="bigps", bufs=2, space="PSUM"))
            ops_ = ctx.enter_context(tc.tile_pool(name="ops", bufs=2, space="PSUM"))
            rps_ = ctx.enter_context(tc.tile_pool(name="rps", bufs=2, space="PSUM"))

            def cload(name, shape, dt):
                t = consts.tile(list(shape), dt, tag=name)
                nc.sync.dma_start(t[:], a[name])
                return t

            perm_t = cload("perm", (P, P), f32r)
            mb = {1: cload("mb1", (P, NKC), f32), 2: cload("mb2", (P, NKC), f32)}
            cq = {n: cload(n, (P, S), bf16) for n in ("cq1", "sq1", "cq2", "sq2")}
            ck = {n: cload(n, (P, SKP), bf16) for n in ("ck1", "sk1", "ck2", "sk2")}
            bqk = {n: cload("b" + n, (P, 4), f32) for n in ("q1", "q2", "k1", "k2")}
            bv = {n: cload("b" + n, (P, EG), f32) for n in ("v1", "v2")}
            ones_bf = consts.tile([P, 1], bf16, tag="ones_bf")
            nc.vector.memset(ones_bf[:], 1.0)
            selmat = cload("selmat", (P, P), f32r)
            zrc = [consts.tile([P, 512], f32r, tag=f"zrc{i}", name=f"zrc{i}")
                   for i in range(2)]
            for i in range(2):
                nc.sync.dma_start(zrc[i][:], a["zeros"])

            def body(_=None):
                # ---------------- projections ----------------
                def proj_qk(dst, x_ap, wname, SF, cos_t, sin_t):
                    """dst: qkv tile [P, 4, SF] f32r (rot-grouped, transposed)."""
                    w_t = wpool.tile([P, KE, EG], f32r, tag="w")
                    nc.sync.dma_start(
                        w_t[:], a["W" + wname].rearrange("(ko p) m -> p ko m", p=P)
                    )
                    slices = [(0, 512)] + ([(512, SF)] if SF > 512 else [])
                    for mp in range(2):  # m-chunk pairs share x chunk loads
                        pss = [bigps.tile([P, 1024], f32, tag="bp", name=f"pss{_i}") for _i in range(2)]
                        for ki in range(KE):
                            xc = xch.tile([P, 1024], f32r, tag="xc")
                            nc.sync.dma_start(
                                xc[:, :SF], x_ap[P * ki : P * (ki + 1), :]
                            )
                            for mi2 in range(2):
                                mi = 2 * mp + mi2
                                for lo, hi in slices:
                                    nc.tensor.matmul(
                                        pss[mi2][:, lo:hi],
                                        w_t[:, ki, ts(mi, P)],
                                        xc[:, lo:hi],
                                        start=(ki == 0),
                                        stop=(ki == KE - 1),
                                    )
                        for mi2 in range(2):
                            mi = 2 * mp + mi2
                            nc.vector.tensor_scalar_add(
                                dst[:, mi, :], pss[mi2][:, :SF], bqk[wname][:, mi : mi + 1]
                            )
                    # rope on rot chunks 0,1
                    for c in range(2):
                        for lo, hi in slices:
                            pp = ops_.tile([P, 512], f32, tag="o", name="pp")[:, : hi - lo]
                            nc.tensor.matmul(
                                pp, perm_t[:], dst[:, c, lo:hi], start=True, stop=True
                            )
                            tmp = sm.tile([P, 512], f32, tag="tmp", name="tmp")[:, : hi - lo]
                            nc.vector.tensor_mul(tmp, pp, sin_t[:, lo:hi])
                            nc.vector.tensor_mul(
                                dst[:, c, lo:hi], dst[:, c, lo:hi], cos_t[:, lo:hi]
                            )
                            nc.vector.tensor_add(dst[:, c, lo:hi], dst[:, c, lo:hi], tmp)

                def proj_v(dst, xp_t, wname):
                    """dst: v tile [P, NKC, EG] bf16 (natural layout)."""
                    w_t = wpool.tile([P, KE, EG], f32r, tag="w")
                    nc.sync.dma_start(
                        w_t[:], a["W" + wname].rearrange("(ko p) m -> p ko m", p=P)
                    )
                    for mi in range(NKC):
                        ps = ops_.tile([P, 512], f32, tag="o")
                        for ki in range(KE):
                            nc.tensor.matmul(
                                ps[:],
                                xp_t[:, ki, ts(mi, P)],
                                w_t[:, ki, :],
                                start=(ki == 0),
                                stop=(ki == KE - 1),
                            )
                        nc.vector.tensor_add(dst[:, mi, :], ps[:], bv[wname][:])

                qT1 = qkv.tile([P, 4, S], f32r, tag="qT1")
                qT2 = qkv.tile([P, 4, S], f32r, tag="qT2")
                kT1 = qkv.tile([P, 4, SKP], f32r, tag="kT1")
                kT2 = qkv.tile([P, 4, SKP], f32r, tag="kT2")
                v1 = qkv.tile([P, NKC, EG], bf16, tag="v1")
                v2 = qkv.tile([P, NKC, EG], bf16, tag="v2")
                att1 = attp.tile([P, 4, S], bf16, tag="att1")
                att2 = attp.tile([P, 4, S], bf16, tag="att2")

                # ---------------- attention ----------------
                def attention(att_t, first, qT, kT, v_t, mbias):
                    for sq in range(2):  # 512-wide query blocks
                        for quad in range(2):  # heads 4*quad..4*quad+3
                            r_ps = rps_.tile([P, 512], f32, tag="r")
                            o_ps = [ops_.tile([P, 512], f32, tag="o", name=f"ops{_i}") for _i in range(2)]
                            uts = None
                            for ki in range(NKC):
                                uts = []
                                for pair in range(2):
                                    st = bigps.tile([P, 1024], f32, tag="bp")
                                    for par in range(2):
                                        j4 = 2 * pair + par
                                        for cc, c in ((quad, 0), (2 + quad, 1)):
                                            nc.tensor.matmul(
                                                st[:, 512 * par : 512 * par + 512],
                                                kT[
                                                    32 * j4 : 32 * j4 + 32,
                                                    cc,
                                                    ts(ki, P),
                                                ],
                                                qT[
                                                    32 * j4 : 32 * j4 + 32,
                                                    cc,
                                                    ts(sq, 512),
                                                ],
                                                start=(c == 0),
                                                stop=(c == 1),
                                                tile_position=(32 * j4, 0),
                                            )
                                    ut = upool.tile([P, 1024], bf16, tag="ut")
                                    nc.scalar.activation(
                                        ut[:],
                                        st[:],
                                        Exp,
                                        bias=mbias[:, ki : ki + 1],
                                        scale=SCALE,
                                    )
                                    uts.append(ut)
                                    # O^T accumulation for this pair
                                    for par in range(2):
                                        h = 4 * quad + 2 * pair + par
                                        nc.tensor.matmul(
                                            o_ps[pair][64 * par : 64 * par + 64, :],
                                            v_t[:, ki, 64 * h : 64 * h + 64],
                                            ut[:, 512 * par : 512 * par + 512],
                                            start=(ki == 0),
                                            stop=(ki == NKC - 1),
                                            tile_position=(0, 64 * par),
                                        )
                                    # r accumulation
                                    for par in range(2):
                                        j4 = 2 * pair + par
                                        nc.tensor.matmul(
                                            r_ps[32 * j4 : 32 * j4 + 1, :],
                                            ones_bf[:, 0:1],
                                            ut[:, 512 * par : 512 * par + 512],
                                            start=(ki == 0),
                                            stop=(ki == NKC - 1),
                                            tile_position=(0, 32 * j4),
                                        )
                            # normalize
                            rc = sm.tile([P, 512], f32, tag="rc")
                            nc.vector.reciprocal_approx_fast(rc[:], r_ps[:])
                            rcr = sm.tile([P, 512], f32r, tag="rcr")
                            nc.vector.tensor_copy(rcr[:], rc[:])
                            # stage each pair's two recip rows at partitions
                            # 0/64 of a zeroed tile; selmat matmul replicates
                            # them over the pair's 64-partition halves
                            for pair in range(2):
                                nc.sync.dma_start(
                                    zrc[pair][0:1, :],
                                    rcr[64 * pair : 64 * pair + 1, :],
                                )
                                nc.sync.dma_start(
                                    zrc[pair][64:65, :],
                                    rcr[64 * pair + 32 : 64 * pair + 33, :],
                                )
                            for pair in range(2):
                                osb = sm.tile([P, 512], f32, tag="osb")
                                nc.vector.tensor_copy(osb[:], o_ps[pair][:])
                                rrep = ops_.tile([P, 512], f32, tag="o")
                                nc.tensor.matmul(
                                    rrep[:], selmat[:], zrc[pair][:],
                                    start=True, stop=True,
                                )
                                chunk = 2 * quad + pair
                                dst = att_t[:, chunk, ts(sq, 512)]
                                if first:
                                    nc.vector.tensor_mul(dst, osb[:], rrep[:])
                                else:
                                    tmpo = sm.tile([P, 512], bf16, tag="tmpo")
                                    nc.vector.tensor_mul(tmpo[:], osb[:], rrep[:])
                                    nc.vector.tensor_add(dst, dst, tmpo[:])

                # ---------------- output projection ----------------
                def outproj(att_t, wo_name, out_ap):
                    wo_t = wpool.tile([P, 4, S], bf16, tag="w")
                    nc.sync.dma_start(
                        wo_t[:], a[wo_name].rearrange("(ko p) f -> p ko f", p=P)
                    )
                    for si in range(8):
                        ps = bigps.tile([P, 1024], f32, tag="bp")
                        for ei in range(4):
                            for fb in range(2):
                                nc.tensor.matmul(
                                    ps[:, 512 * fb : 512 * fb + 512],
                                    att_t[:, ei, ts(si, P)],
                                    wo_t[:, ei, 512 * fb : 512 * fb + 512],
                                    start=(ei == 0),
                                    stop=(ei == 3),
                                )
                        osb = outp.tile([P, 1024], f32, tag="ob")
                        nc.vector.tensor_copy(osb[:], ps[:])
                        nc.sync.dma_start(out_ap[ts(si, P), :], osb[:])

                # emission order interleaves projections with attention combos
                proj_qk(qT1, a["x1T"], "q1", S, cq["cq1"], cq["sq1"])
                xp2 = xpp.tile([P, KE, SKP], f32r, tag="xp")
                nc.sync.dma_start(xp2[:], a["x2p"].rearrange("(ko p) s -> p ko s", p=P))
                proj_qk(kT2, a["x2p"], "k2", SKP, ck["ck2"], ck["sk2"])
                proj_v(v2, xp2, "v2")
                attention(att1, True, qT1, kT2, v2, mb[2])
                xp1 = xpp.tile([P, KE, SKP], f32r, tag="xp")
                nc.sync.dma_start(xp1[:], a["x1p"].rearrange("(ko p) s -> p ko s", p=P))
                proj_qk(kT1, a["x1p"], "k1", SKP, ck["ck1"], ck["sk1"])
                proj_v(v1, xp1, "v1")
                attention(att1, False, qT1, kT1, v1, mb[1])
                proj_qk(qT2, a["x2T"], "q2", S, cq["cq2"], cq["sq2"])
                attention(att2, True, qT2, kT1, v1, mb[1])
                attention(att2, False, qT2, kT2, v2, mb[2])
                outproj(att1, "Wo1", out1)
                outproj(att2, "Wo2", out2)

            if reps > 1:
                with tc.For_i(0, reps, 1):
                    body()
            else:
                body()

    with tile.TileContext(nc) as tc:
        emit(tc)
    nc.compile()
    _PROG_CACHE[key] = nc
    return nc


def _prep_inputs(inputs):
    """Host-side sharding/packing. Returns (in_maps, SKP, bo1, bo2)."""
    f32 = np.float32
    x1 = np.asarray(inputs["x1"], f32)
    x2 = np.asarray(inputs["x2"], f32)
    m1 = np.asarray(inputs["x1_padding_mask"]).astype(np.int64)
    m2 = np.asarray(inputs["x2_padding_mask"]).astype(np.int64)
    cos1 = np.asarray(inputs["cos1"], f32).reshape(S, R)
    sin1 = np.asarray(inputs["sin1"], f32).reshape(S, R)
    cos2 = np.asarray(inputs["cos2"], f32).reshape(S, R)
    sin2 = np.asarray(inputs["sin2"], f32).reshape(S, R)

    idx1 = [np.nonzero(m1[b])[0] for b in range(B)]
    idx2 = [np.nonzero(m2[b])[0] for b in range(B)]
    maxn = max(
        [len(i) for i in idx1] + [len(i) for i in idx2] + [1]
    )
    SKP = ((maxn + P - 1) // P) * P
    NKC = SKP // P
    perm = _rot_perm()

    import ml_dtypes

    bf16 = ml_dtypes.bfloat16

    # per-group weight tensors (shared by the 4 cores of a group)
    gw = []
    for g in range(2):
        rows = slice(g * EG, (g + 1) * EG)
        d = {}
        for n in ("q1", "q2", "k1", "k2"):
            W = np.asarray(inputs["W" + n], f32)[rows][perm]
            bb = np.asarray(inputs["b" + n], f32)[rows][perm]
            d["W" + n] = np.ascontiguousarray(W.T)
            d["b" + n] = np.ascontiguousarray(bb.reshape(4, P).T)
        for n in ("v1", "v2"):
            W = np.asarray(inputs["W" + n], f32)[rows]
            bb = np.asarray(inputs["b" + n], f32)[rows]
            d["W" + n] = np.ascontiguousarray(W.T)
            d["b" + n] = np.ascontiguousarray(
                np.broadcast_to(bb[None, :], (P, EG))
            )
        for n in ("o1", "o2"):
            W = np.asarray(inputs["W" + n], f32)[:, rows]
            d["W" + n] = np.ascontiguousarray(W.T).astype(bf16)
        gw.append(d)

    cq = {
        "cq1": np.tile(cos1.T, (4, 1)).astype(bf16),
        "sq1": np.tile(sin1.T, (4, 1)).astype(bf16),
        "cq2": np.tile(cos2.T, (4, 1)).astype(bf16),
        "sq2": np.tile(sin2.T, (4, 1)).astype(bf16),
    }

    in_maps = []
    for c in range(NCORES):
        b, g = c // 2, c % 2
        m = {}
        m["x1T"] = np.ascontiguousarray(x1[b].T)
        m["x2T"] = np.ascontiguousarray(x2[b].T)
        for which, xb, idx, cos, sin in (
            (1, x1[b], idx1[b], cos1, sin1),
            (2, x2[b], idx2[b], cos2, sin2),
        ):
            n = len(idx)
            xp = np.zeros((SKP, E), f32)
            xp[:n] = xb[idx]
            m[f"x{which}p"] = np.ascontiguousarray(xp.T)
            ckk = np.zeros((R, SKP), f32)
            skk = np.zeros((R, SKP), f32)
            ckk[:, :n] = cos.T[:, idx]
            skk[:, :n] = sin.T[:, idx]
            m[f"ck{which}"] = np.tile(ckk, (4, 1)).astype(bf16)
            m[f"sk{which}"] = np.tile(skk, (4, 1)).astype(bf16)
            mbv = np.full(SKP, MASK_BIAS, f32)
            mbv[:n] = 0.0
            m[f"mb{which}"] = np.ascontiguousarray(mbv.reshape(NKC, P).T)
        for n in ("q1", "q2", "k1", "k2", "v1", "v2"):
            m["W" + n] = gw[g]["W" + n]
            m["b" + n] = gw[g]["b" + n]
        m["Wo1"] = gw[g]["Wo1"]
        m["Wo2"] = gw[g]["Wo2"]
        m.update(cq)
        # rotate-half permutation matrix (block-diag over 4x 32-dim groups)
        pm = np.zeros((P, P), f32)
        for blk in range(4):
            o = 32 * blk
            for i in range(16):
                pm[o + 16 + i, o + i] = -1.0
                pm[o + i, o + 16 + i] = 1.0
        m["perm"] = pm
        sel = np.zeros((P, P), f32)
        sel[0, 0:64] = 1.0
        sel[64, 64:128] = 1.0
        m["selmat"] = sel
        m["zeros"] = np.zeros((P, 512), f32)
        in_maps.append(m)

    bo1 = np.asarray(inputs["bo1"], f32)
    bo2 = np.asarray(inputs["bo2"], f32)
    return in_maps, SKP, bo1, bo2


def kernel(**inputs):
    from concourse.bass_utils import run_bass_kernel_spmd

    in_maps, SKP, bo1, bo2 = _prep_inputs(inputs)
    nc = _build_program(SKP)
    res = run_bass_kernel_spmd(nc, in_maps, core_ids=list(range(NCORES)))
    o1 = np.stack(
        [res.results[2 * b]["o1"] + res.results[2 * b + 1]["o1"] + bo1 for b in range(B)]
    )
    o2 = np.stack(
        [res.results[2 * b]["o2"] + res.results[2 * b + 1]["o2"] + bo2 for b in range(B)]
    )
    return o1.astype(np.float32), o2.astype(np.float32)


# revision 8
# speedup vs baseline: 1.6417x; 1.6417x over previous
"""DualAttention (cross+self bidirectional attention, 2 streams) on 8 TRN2 cores.

Sharding: data-parallel over batch (4) x tensor-parallel over heads (2 groups
of 8 heads). Core c handles batch c//2, head-group c%2. Each core computes its
head-group's slice of all 6 input projections, RoPE, the 4 attention combos,
and a partial output projection; the host sums the two partial out-projections
per batch and adds the output bias.

Key device-side layout choices:
  - q/k projections computed TRANSPOSED ([e_out, s]) with rot-grouped row
    permutation so RoPE's rotate_half becomes a fixed 128x128 permutation
    matmul + elementwise combines, and attention scores can be computed as
    S^T = K^T(d,sk)^T-contracted-with-Q^T(d,sq) using K=32 row-packed matmuls.
  - keys are pre-packed on the host (masked keys dropped, padded to SKP), so
    softmax masking reduces to a per-partition bias of -30000 on pad rows and
    the whole attention is ~2x smaller.
  - softmax is unnormalized: U = exp(scale*S^T + maskbias); row sums r come
    from ones-lhsT matmuls; normalization multiplies O^T by replicated 1/r.
  - matmuls run in float32r (tf32-class, full PE rate); U/V/att are bf16.
"""

import numpy as np

B, S, E, H = 4, 1024, 1024, 16
D, R = 64, 32
HG, EG = 8, 512  # heads / e-columns per head-group
P = 128
KE = E // P  # contraction chunks of a projection
SCALE = D ** -0.5
NCORES = 8
MASK_BIAS = -30000.0

_PROG_CACHE = {}


def _rot_perm():
    """Row order: [h0..h3 rot | h4..h7 rot | h0..h3 pass | h4..h7 pass]."""
    idx = []
    for blk in range(2):
        for j in range(4):
            h = 4 * blk + j
            idx += [64 * h + d for d in range(32)]
    for blk in range(2):
        for j in range(4):
            h = 4 * blk + j
            idx += [64 * h + d for d in range(32, 64)]
    return np.array(idx, np.int64)


def _build_program(SKP, reps=1):
    key = (SKP, reps)
    if key in _PROG_CACHE:
        return _PROG_CACHE[key]

    import concourse.bass as bass
    import concourse.tile as tile
    from concourse import bacc, mybir
    from contextlib import ExitStack

    f32 = mybir.dt.float32
    f32r = mybir.dt.float32r
    bf16 = mybir.dt.bfloat16
    NKC = SKP // P
    ts = bass.ts

    nc = bacc.Bacc("TRN2", target_bir_lowering=False, debug=False, num_devices=NCORES)

    def din(name, shape, dt):
        return nc.dram_tensor(name, list(shape), dt, kind="ExternalInput").ap()

    a = {}
    a["x1T"] = din("x1T", (E, S), f32r)
    a["x2T"] = din("x2T", (E, S), f32r)
    a["x1p"] = din("x1p", (E, SKP), f32r)
    a["x2p"] = din("x2p", (E, SKP), f32r)
    for n in ("q1", "q2", "k1", "k2", "v1", "v2"):
        a["W" + n] = din("W" + n, (E, EG), f32r)
    a["Wo1"] = din("Wo1", (EG, S), bf16)
    a["Wo2"] = din("Wo2", (EG, S), bf16)
    for n in ("q1", "q2", "k1", "k2"):
        a["b" + n] = din("b" + n, (P, 4), f32)
    a["bv1"] = din("bv1", (P, EG), f32)
    a["bv2"] = din("bv2", (P, EG), f32)
    for n in ("cq1", "sq1", "cq2", "sq2"):
        a[n] = din(n, (P, S), bf16)
    for n in ("ck1", "sk1", "ck2", "sk2"):
        a[n] = din(n, (P, SKP), bf16)
    a["mb1"] = din("mb1", (P, NKC), f32)
    a["mb2"] = din("mb2", (P, NKC), f32)
    a["perm"] = din("perm", (P, P), f32r)
    a["selmat"] = din("selmat", (P, P), f32r)
    a["zeros"] = din("zeros", (P, 512), f32r)
    out1 = nc.dram_tensor("o1", [S, E], f32, kind="ExternalOutput").ap()
    out2 = nc.dram_tensor("o2", [S, E], f32, kind="ExternalOutput").ap()

    Exp = mybir.ActivationFunctionType.Exp

    def emit(tc):
        with ExitStack() as ctx:
            consts = ctx.enter_context(tc.tile_pool(name="consts", bufs=1))
            xch = ctx.enter_context(tc.tile_pool(name="xch", bufs=3))
            xpp = ctx.enter_context(tc.tile_pool(name="xpp", bufs=1))
            wpool = ctx.enter_context(tc.tile_pool(name="wpool", bufs=2))
            qkv = ctx.enter_context(tc.tile_pool(name="qkv", bufs=1))
            attp = ctx.enter_context(tc.tile_pool(name="attp", bufs=1))
            upool = ctx.enter_context(tc.tile_pool(name="upool", bufs=3))
            sm = ctx.enter_context(tc.tile_pool(name="sm", bufs=2))
            outp = ctx.enter_context(tc.tile_pool(name="outp", bufs=2))
            bigps = ctx.enter_context(tc.tile_pool(name="bigps", bufs=2, space="PSUM"))
            ops_ = ctx.enter_context(tc.tile_pool(name="ops", bufs=2, space="PSUM"))
            rps_ = ctx.enter_context(tc.tile_pool(name="rps", bufs=2, space="PSUM"))

            def cload(name, shape, dt):
                t = consts.tile(list(shape), dt, tag=name)
                nc.sync.dma_start(t[:], a[name])
                return t

            perm_t = cload("perm", (P, P), f32r)
            mb = {1: cload("mb1", (P, NKC), f32), 2: cload("mb2", (P, NKC), f32)}
            cq = {n: cload(n, (P, S), bf16) for n in ("cq1", "sq1", "cq2", "sq2")}
            ck = {n: cload(n, (P, SKP), bf16) for n in ("ck1", "sk1", "ck2", "sk2")}
            bqk = {n: cload("b" + n, (P, 4), f32) for n in ("q1", "q2", "k1", "k2")}
            bv = {n: cload("b" + n, (P, EG), f32) for n in ("v1", "v2")}
            ones_bf = consts.tile([P, 1], bf16, tag="ones_bf")
            nc.vector.memset(ones_bf[:], 1.0)
            selmat = cload("selmat", (P, P), f32r)
            zrc = [consts.tile([P, 512], f32r, tag=f"zrc{i}", name=f"zrc{i}")
                   for i in range(2)]
            for i in range(2):
                nc.gpsimd.dma_start(zrc[i][:], a["zeros"])

            def body(_=None):
                # ---------------- projections ----------------
                wq_ctr = [0]

                def _wdma(w_t, ap3):
                    eng = (nc.scalar, nc.sync)[wq_ctr[0] % 2]
                    wq_ctr[0] += 1
                    eng.dma_start(w_t[:], ap3)

                def proj_qk(dst, x_ap, wname, SF, cos_t, sin_t):
                    """dst: qkv tile [P, 4, SF] f32r (rot-grouped, transposed)."""
                    w_t = wpool.tile([P, KE, EG], f32r, tag="w")
                    _wdma(w_t, a["W" + wname].rearrange("(ko p) m -> p ko m", p=P))
                    slices = [(lo, min(lo + 512, SF)) for lo in range(0, SF, 512)]
                    for mp in range(2):  # m-chunk pairs share x chunk loads
                        pss = [bigps.tile([P, 1024], f32, tag="bp", name=f"pss{_i}") for _i in range(2)]
                        for ki in range(KE):
                            xc = xch.tile([P, 1024], f32r, tag="xc")
                            dma_eng = nc.sync if ki % 2 == 0 else nc.scalar
                            dma_eng.dma_start(
                                xc[:, :SF], x_ap[P * ki : P * (ki + 1), :]
                            )
                            for mi2 in range(2):
                                mi = 2 * mp + mi2
                                for lo, hi in slices:
                                    nc.tensor.matmul(
                                        pss[mi2][:, lo:hi],
                                        w_t[:, ki, ts(mi, P)],
                                        xc[:, lo:hi],
                                        start=(ki == 0),
                                        stop=(ki == KE - 1),
                                    )
                        for mi2 in range(2):
                            mi = 2 * mp + mi2
                            nc.vector.tensor_scalar_add(
                                dst[:, mi, :], pss[mi2][:, :SF], bqk[wname][:, mi : mi + 1]
                            )
                    # rope on rot chunks 0,1
                    for c in range(2):
                        for lo, hi in slices:
                            pp = ops_.tile([P, 512], f32, tag="o", name="pp")[:, : hi - lo]
                            nc.tensor.matmul(
                                pp, perm_t[:], dst[:, c, lo:hi], start=True, stop=True
                            )
                            tmp = sm.tile([P, 512], f32, tag="tmp", name="tmp")[:, : hi - lo]
                            nc.vector.tensor_mul(tmp, pp, sin_t[:, lo:hi])
                            nc.vector.tensor_mul(
                                dst[:, c, lo:hi], dst[:, c, lo:hi], cos_t[:, lo:hi]
                            )
                            nc.vector.tensor_add(dst[:, c, lo:hi], dst[:, c, lo:hi], tmp)

                def proj_v(dst, xp_t, wname):
                    """dst: v tile [P, NKC, EG] bf16 (natural layout)."""
                    w_t = wpool.tile([P, KE, EG], f32r, tag="w")
                    _wdma(w_t, a["W" + wname].rearrange("(ko p) m -> p ko m", p=P))
                    for mi in range(NKC):
                        ps = ops_.tile([P, 512], f32, tag="o")
                        for ki in range(KE):
                            nc.tensor.matmul(
                                ps[:],
                                xp_t[:, ki, ts(mi, P)],
                                w_t[:, ki, :],
                                start=(ki == 0),
                                stop=(ki == KE - 1),
                            )
                        nc.vector.tensor_add(dst[:, mi, :], ps[:], bv[wname][:])

                qT1 = qkv.tile([P, 4, S], f32r, tag="qT1")
                qT2 = qkv.tile([P, 4, S], f32r, tag="qT2")
                kT1 = qkv.tile([P, 4, SKP], f32r, tag="kT1")
                kT2 = qkv.tile([P, 4, SKP], f32r, tag="kT2")
                v1 = qkv.tile([P, NKC, EG], bf16, tag="v1")
                v2 = qkv.tile([P, NKC, EG], bf16, tag="v2")
                att1 = attp.tile([P, 4, S], bf16, tag="att1")
                att2 = attp.tile([P, 4, S], bf16, tag="att2")

                # ---------------- attention ----------------
                def attention(att_t, first, qT, kT, v_t, mbias):
                    for sq in range(2):  # 512-wide query blocks
                        for quad in range(2):  # heads 4*quad..4*quad+3
                            r_ps = rps_.tile([P, 512], f32, tag="r")
                            o_ps = [ops_.tile([P, 512], f32, tag="o", name=f"ops{_i}") for _i in range(2)]
                            uts = None
                            for ki in range(NKC):
                                uts = []
                                for pair in range(2):
                                    st = bigps.tile([P, 1024], f32, tag="bp")
                                    for par in range(2):
                                        j4 = 2 * pair + par
                                        for cc, c in ((quad, 0), (2 + quad, 1)):
                                            nc.tensor.matmul(
                                                st[:, 512 * par : 512 * par + 512],
                                                kT[
                                                    32 * j4 : 32 * j4 + 32,
                                                    cc,
                                                    ts(ki, P),
                                                ],
                                                qT[
                                                    32 * j4 : 32 * j4 + 32,
                                                    cc,
                                                    ts(sq, 512),
                                                ],
                                                start=(c == 0),
                                                stop=(c == 1),
                                                tile_position=(32 * j4, 0),
                                            )
                                    ut = upool.tile([P, 1024], bf16, tag="ut")
                                    nc.scalar.activation(
                                        ut[:],
                                        st[:],
                                        Exp,
                                        bias=mbias[:, ki : ki + 1],
                                        scale=SCALE,
                                    )
                                    uts.append(ut)
                                    # O^T accumulation for this pair
                                    for par in range(2):
                                        h = 4 * quad + 2 * pair + par
                                        nc.tensor.matmul(
                                            o_ps[pair][64 * par : 64 * par + 64, :],
                                            v_t[:, ki, 64 * h : 64 * h + 64],
                                            ut[:, 512 * par : 512 * par + 512],
                                            start=(ki == 0),
                                            stop=(ki == NKC - 1),
                                            tile_position=(0, 64 * par),
                                        )
                                    # r accumulation
                                    for par in range(2):
                                        j4 = 2 * pair + par
                                        nc.tensor.matmul(
                                            r_ps[32 * j4 : 32 * j4 + 1, :],
                                            ones_bf[:, 0:1],
                                            ut[:, 512 * par : 512 * par + 512],
                                            start=(ki == 0),
                                            stop=(ki == NKC - 1),
                                            tile_position=(0, 32 * j4),
                                        )
                            # normalize
                            rc = sm.tile([P, 512], f32, tag="rc")
                            nc.vector.reciprocal_approx_fast(rc[:], r_ps[:])
                            rcr = sm.tile([P, 512], f32r, tag="rcr")
                            nc.vector.tensor_copy(rcr[:], rc[:])
                            # stage each pair's two recip rows at partitions
                            # 0/64 of a zeroed tile; selmat matmul replicates
                            # them over the pair's 64-partition halves
                            for pair in range(2):
                                nc.gpsimd.dma_start(
                                    zrc[pair][0:1, :],
                                    rcr[64 * pair : 64 * pair + 1, :],
                                )
                                nc.gpsimd.dma_start(
                                    zrc[pair][64:65, :],
                                    rcr[64 * pair + 32 : 64 * pair + 33, :],
                                )
                            for pair in range(2):
                                osb = sm.tile([P, 512], f32, tag="osb")
                                nc.vector.tensor_copy(osb[:], o_ps[pair][:])
                                rrep = ops_.tile([P, 512], f32, tag="o")
                                nc.tensor.matmul(
                                    rrep[:], selmat[:], zrc[pair][:],
                                    start=True, stop=True,
                                )
                                chunk = 2 * quad + pair
                                dst = att_t[:, chunk, ts(sq, 512)]
                                if first:
                                    nc.vector.tensor_mul(dst, osb[:], rrep[:])
                                else:
                                    tmpo = sm.tile([P, 512], bf16, tag="tmpo")
                                    nc.vector.tensor_mul(tmpo[:], osb[:], rrep[:])
                                    nc.vector.tensor_add(dst, dst, tmpo[:])

                # ---------------- output projection ----------------
                def outproj(att_t, wo_name, out_ap):
                    wo_t = wpool.tile([P, 4, S], bf16, tag="w")
                    _wdma(wo_t, a[wo_name].rearrange("(ko p) f -> p ko f", p=P))
                    for si in range(8):
                        ps = bigps.tile([P, 1024], f32, tag="bp")
                        for ei in range(4):
                            for fb in range(2):
                                nc.tensor.matmul(
                                    ps[:, 512 * fb : 512 * fb + 512],
                                    att_t[:, ei, ts(si, P)],
                                    wo_t[:, ei, 512 * fb : 512 * fb + 512],
                                    start=(ei == 0),
                                    stop=(ei == 3),
                                )
                        osb = outp.tile([P, 1024], f32, tag="ob")
                        nc.vector.tensor_copy(osb[:], ps[:])
                        eng = nc.sync if si % 2 == 0 else nc.scalar
                        eng.dma_start(out_ap[ts(si, P), :], osb[:])

                # emission order interleaves projections with attention combos
                proj_qk(qT1, a["x1T"], "q1", S, cq["cq1"], cq["sq1"])
                xp2 = xpp.tile([P, KE, SKP], f32r, tag="xp")
                nc.gpsimd.dma_start(xp2[:], a["x2p"].rearrange("(ko p) s -> p ko s", p=P))
                proj_qk(kT2, a["x2p"], "k2", SKP, ck["ck2"], ck["sk2"])
                proj_v(v2, xp2, "v2")
                attention(att1, True, qT1, kT2, v2, mb[2])
                xp1 = xpp.tile([P, KE, SKP], f32r, tag="xp")
                nc.gpsimd.dma_start(xp1[:], a["x1p"].rearrange("(ko p) s -> p ko s", p=P))
                proj_qk(kT1, a["x1p"], "k1", SKP, ck["ck1"], ck["sk1"])
                proj_v(v1, xp1, "v1")
                attention(att1, False, qT1, kT1, v1, mb[1])
                proj_qk(qT2, a["x2T"], "q2", S, cq["cq2"], cq["sq2"])
                attention(att2, True, qT2, kT1, v1, mb[1])
                attention(att2, False, qT2, kT2, v2, mb[2])
                outproj(att1, "Wo1", out1)
                outproj(att2, "Wo2", out2)

            if reps > 1:
                with tc.For_i(0, reps, 1):
                    body()
            else:
                body()

    with tile.TileContext(nc) as tc:
        emit(tc)
    nc.compile()
    _PROG_CACHE[key] = nc
    return nc


def _prep_inputs(inputs):
    """Host-side sharding/packing. Returns (in_maps, SKP, bo1, bo2)."""
    f32 = np.float32
    x1 = np.asarray(inputs["x1"], f32)
    x2 = np.asarray(inputs["x2"], f32)
    m1 = np.asarray(inputs["x1_padding_mask"]).astype(np.int64)
    m2 = np.asarray(inputs["x2_padding_mask"]).astype(np.int64)
    cos1 = np.asarray(inputs["cos1"], f32).reshape(S, R)
    sin1 = np.asarray(inputs["sin1"], f32).reshape(S, R)
    cos2 = np.asarray(inputs["cos2"], f32).reshape(S, R)
    sin2 = np.asarray(inputs["sin2"], f32).reshape(S, R)

    idx1 = [np.nonzero(m1[b])[0] for b in range(B)]
    idx2 = [np.nonzero(m2[b])[0] for b in range(B)]
    maxn = max(
        [len(i) for i in idx1] + [len(i) for i in idx2] + [1]
    )
    SKP = ((maxn + P - 1) // P) * P
    NKC = SKP // P
    perm = _rot_perm()

    import ml_dtypes

    bf16 = ml_dtypes.bfloat16

    # per-group weight tensors (shared by the 4 cores of a group)
    gw = []
    for g in range(2):
        rows = slice(g * EG, (g + 1) * EG)
        d = {}
        for n in ("q1", "q2", "k1", "k2"):
            W = np.asarray(inputs["W" + n], f32)[rows][perm]
            bb = np.asarray(inputs["b" + n], f32)[rows][perm]
            d["W" + n] = np.ascontiguousarray(W.T)
            d["b" + n] = np.ascontiguousarray(bb.reshape(4, P).T)
        for n in ("v1", "v2"):
            W = np.asarray(inputs["W" + n], f32)[rows]
            bb = np.asarray(inputs["b" + n], f32)[rows]
            d["W" + n] = np.ascontiguousarray(W.T)
            d["b" + n] = np.ascontiguousarray(
                np.broadcast_to(bb[None, :], (P, EG))
            )
        for n in ("o1", "o2"):
            W = np.asarray(inputs["W" + n], f32)[:, rows]
            d["W" + n] = np.ascontiguousarray(W.T).astype(bf16)
        gw.append(d)

    cq = {
        "cq1": np.tile(cos1.T, (4, 1)).astype(bf16),
        "sq1": np.tile(sin1.T, (4, 1)).astype(bf16),
        "cq2": np.tile(cos2.T, (4, 1)).astype(bf16),
        "sq2": np.tile(sin2.T, (4, 1)).astype(bf16),
    }

    in_maps = []
    for c in range(NCORES):
        b, g = c // 2, c % 2
        m = {}
        m["x1T"] = np.ascontiguousarray(x1[b].T)
        m["x2T"] = np.ascontiguousarray(x2[b].T)
        for which, xb, idx, cos, sin in (
            (1, x1[b], idx1[b], cos1, sin1),
            (2, x2[b], idx2[b], cos2, sin2),
        ):
            n = len(idx)
            xp = np.zeros((SKP, E), f32)
            xp[:n] = xb[idx]
            m[f"x{which}p"] = np.ascontiguousarray(xp.T)
            ckk = np.zeros((R, SKP), f32)
            skk = np.zeros((R, SKP), f32)
            ckk[:, :n] = cos.T[:, idx]
            skk[:, :n] = sin.T[:, idx]
            m[f"ck{which}"] = np.tile(ckk, (4, 1)).astype(bf16)
            m[f"sk{which}"] = np.tile(skk, (4, 1)).astype(bf16)
            mbv = np.full(SKP, MASK_BIAS, f32)
            mbv[:n] = 0.0
            m[f"mb{which}"] = np.ascontiguousarray(mbv.reshape(NKC, P).T)
        for n in ("q1", "q2", "k1", "k2", "v1", "v2"):
            m["W" + n] = gw[g]["W" + n]
            m["b" + n] = gw[g]["b" + n]
        m["Wo1"] = gw[g]["Wo1"]
        m["Wo2"] = gw[g]["Wo2"]
        m.update(cq)
        # rotate-half permutation matrix (block-diag over 4x 32-dim groups)
        pm = np.zeros((P, P), f32)
        for blk in range(4):
            o = 32 * blk
            for i in range(16):
                pm[o + 16 + i, o + i] = -1.0
                pm[o + i, o + 16 + i] = 1.0
        m["perm"] = pm
        sel = np.zeros((P, P), f32)
        sel[0, 0:64] = 1.0
        sel[64, 64:128] = 1.0
        m["selmat"] = sel
        m["zeros"] = np.zeros((P, 512), f32)
        in_maps.append(m)

    bo1 = np.asarray(inputs["bo1"], f32)
    bo2 = np.asarray(inputs["bo2"], f32)
    return in_maps, SKP, bo1, bo2


def kernel(**inputs):
    from concourse.bass_utils import run_bass_kernel_spmd

    in_maps, SKP, bo1, bo2 = _prep_inputs(inputs)
    nc = _build_program(SKP)
    res = run_bass_kernel_spmd(nc, in_maps, core_ids=list(range(NCORES)))
    o1 = np.stack(
        [res.results[2 * b]["o1"] + res.results[2 * b + 1]["o1"] + bo1 for b in range(B)]
    )
    o2 = np.stack(
        [res.results[2 * b]["o2"] + res.results[2 * b + 1]["o2"] + bo2 for b in range(B)]
    )
    return o1.astype(np.float32), o2.astype(np.float32)
